# revision 12
# baseline (speedup 1.0000x reference)
"""Trainium2 Bass kernel for nn_ColorTransform: per-pixel degree-3 polynomial
color transform  y[b,c,h,w] = bias[c] + sum_f weight[f,c] * mono_f(x[b,:,h,w]).

Strategy (pure data parallel over batch across 8 cores; identical SPMD program):

The 3->19->3 per-pixel map is algebraically a degree-<=3 polynomial in the 3
channels. Cubes and squares of 10 FIXED generic affine forms L_i = a_i.x + b_i
span the full 20-dim space of degree-<=3 polynomials in 3 variables, so

    y_c = sum_i cq[i,c] * L_i^3 + cs[i,c] * L_i^2

with (cq, cs) solved at runtime (tiny 20x20 float64 solve) from (weight, bias).

On-chip pipeline per chunk of 12 pixel-groups x 512 pixels (partition-packed,
10 form-rows per group = 120 partitions):
  DMA in  -> X [37,512] fp32 (36 x-rows + const ones row)
  GPSIMD  -> Xh fp16 cast
  PE  M1  -> P1 = lhsT1^T @ Xh (block-diag forms)        [120,512] PSUM
  ACT     -> S = Square(P1)  (fp32r rounded)             [120,512] SBUF
  DVE     -> Q = S * P1      (cube, fp32r)               [120,512] SBUF
  PE  M2  -> P2 = W_q^T @ Q + W_s^T @ S (PSUM accum)     [36,512]  PSUM
  ACT/DVE -> O copy-out (split by columns)               [36,512]  SBUF
  DMA out -> y
"""
import numpy as np
from itertools import product as _product
from math import factorial as _factorial

import concourse.bass as bass
import concourse.tile as tile
from concourse import bacc, mybir
from concourse.bass_utils import run_bass_kernel_spmd

# ---------------------------------------------------------------- constants
B, C, H, W = 16, 3, 512, 512
HW = H * W                 # 262144 pixels per (batch, channel) plane
NCORES = 8
BPC = B // NCORES          # batches per core = 2
N = 512                    # pixel columns per group per chunk
GPB = 6                    # groups per batch in a full chunk
NG = BPC * GPB             # 12 groups per full chunk
R = 10                     # affine forms per group
FULL_CHUNKS = 85           # 85*6*512 = 261120 px per batch plane
TAIL_GPB = 2               # tail: 2 groups per batch (261120 + 2*512 = 262144)
TAIL_NG = BPC * TAIL_GPB   # 4
ACT_COLS = 320             # out-copy column split: ACT does [0,320), DVE the rest

# optimized generic affine forms (see form_opt.py): amplification ~4.7
AV = np.array([
    [ 0.37934126,  0.23092419,  0.89597669],
    [-0.11446939,  0.06385343,  0.99137253],
    [ 0.21945084, -0.83239185,  0.50888617],
    [-0.6455188 , -0.57861811,  0.49850432],
    [-0.02451489,  0.30102502, -0.95330108],
    [ 0.09930513,  0.99370851, -0.0517869 ],
    [-0.56684164,  0.6902054 ,  0.44978558],
    [-0.71569315, -0.69804976,  0.02257986],
    [ 0.94752367, -0.15834609, -0.27771463],
    [ 0.99764591, -0.01721242,  0.06638047],
])
BV = np.array([-0.58237884,  0.03295331,  0.14354757,  0.34220693,  0.78767153,
               -0.00392558, -0.31987566,  1.06484995, -0.16575755,  0.0089387 ])

# monomial order matching the reference poly_feature expansion (degree 1..3)
MONOMIALS = [
    (1,0,0),(0,1,0),(0,0,1),
    (2,0,0),(1,1,0),(1,0,1),(0,2,0),(0,1,1),(0,0,2),
    (3,0,0),(2,1,0),(2,0,1),(1,2,0),(1,1,1),(1,0,2),(0,3,0),(0,2,1),(0,1,2),(0,0,3),
]
ALL_MONO = [(0,0,0)] + MONOMIALS


def _expand(a, b, power):
    coeffs = {}
    for ks in _product(range(power+1), repeat=4):
        if sum(ks) != power:
            continue
        k0, k1, k2, kb = ks
        mult = _factorial(power)/(_factorial(k0)*_factorial(k1)*_factorial(k2)*_factorial(kb))
        coeffs[(k0,k1,k2)] = coeffs.get((k0,k1,k2), 0.0) + \
            mult * a[0]**k0 * a[1]**k1 * a[2]**k2 * b**kb
    return np.array([coeffs.get(m, 0.0) for m in ALL_MONO])


_A_SYS = np.stack(
    [_expand(AV[i], BV[i], 3) for i in range(R)] +
    [_expand(AV[i], BV[i], 2) for i in range(R)], axis=1)          # [20, 20]


def _solve_coeffs(weight, bias):
    """-> cq [R,3], cs [R,3] float32 such that P_c = sum cq L^3 + cs L^2."""
    T = np.zeros((20, 3))
    T[0] = np.asarray(bias, np.float64)
    T[1:] = np.asarray(weight, np.float64)
    Cf = np.linalg.solve(_A_SYS, T)
    return Cf[:R].astype(np.float32), Cf[R:].astype(np.float32)


def _lhs1(ngroups):
    """M1 weights [3*ngroups+1, R*ngroups] fp16: shared ones row 0 + block-diag forms."""
    K = 3 * ngroups + 1
    m = np.zeros((K, R * ngroups), np.float32)
    for g in range(ngroups):
        for i in range(R):
            m[1+g*3:1+(g+1)*3, g*R + i] = AV[i]
            m[0, g*R + i] = BV[i]
    return m.astype(np.float16)


def _lhs2(coeff, ngroups):
    """M2 weights [R*ngroups, 3*ngroups] f32: block-diag runtime coefficients."""
    m = np.zeros((R * ngroups, 3 * ngroups), np.float32)
    for g in range(ngroups):
        m[g*R:(g+1)*R, g*3:(g+1)*3] = coeff
    return m


# ---------------------------------------------------------------- bass build
_NC_CACHE = {}


def build_nc(reps=1):
    if reps in _NC_CACHE:
        return _NC_CACHE[reps]
    f32, f16, f32r = mybir.dt.float32, mybir.dt.float16, mybir.dt.float32r
    nc = bacc.Bacc("TRN2", target_bir_lowering=False, debug=False, num_devices=NCORES)

    xs = nc.dram_tensor("xs", [BPC, C, HW], f16, kind="ExternalInput")
    wm1 = nc.dram_tensor("wm1", [3*NG+1, R*NG], f16, kind="ExternalInput")
    w2q = nc.dram_tensor("w2q", [R*NG, 3*NG], f32, kind="ExternalInput")
    w2s = nc.dram_tensor("w2s", [R*NG, 3*NG], f32, kind="ExternalInput")
    wm1t = nc.dram_tensor("wm1t", [3*TAIL_NG+1, R*TAIL_NG], f16, kind="ExternalInput")
    w2qt = nc.dram_tensor("w2qt", [R*TAIL_NG, 3*TAIL_NG], f32, kind="ExternalInput")
    w2st = nc.dram_tensor("w2st", [R*TAIL_NG, 3*TAIL_NG], f32, kind="ExternalInput")
    y = nc.dram_tensor("y", [BPC, C, HW], f32, kind="ExternalOutput")

    with tile.TileContext(nc) as tc:
        with (
            tc.tile_pool(name="wpool", bufs=1) as wpool,
            tc.tile_pool(name="xpool", bufs=3) as xpool,
            tc.tile_pool(name="spool", bufs=3) as spool,
            tc.tile_pool(name="qpool", bufs=3) as qpool,
            tc.tile_pool(name="opool", bufs=3) as opool,
            tc.tile_pool(name="p1pool", bufs=2, space="PSUM") as p1pool,
            tc.tile_pool(name="p2pool", bufs=2, space="PSUM") as p2pool,
            tc.tile_pool(name="p1tpool", bufs=1, space="PSUM") as p1tpool,
            tc.tile_pool(name="p2tpool", bufs=1, space="PSUM") as p2tpool,
        ):
            # --- load + round weights once
            wm1_sb = wpool.tile([3*NG+1, R*NG], f16, tag="wm1")
            nc.gpsimd.dma_start(wm1_sb[:], wm1[:])
            w2q_sb = wpool.tile([R*NG, 3*NG], f32, tag="w2q")
            nc.gpsimd.dma_start(w2q_sb[:], w2q[:])
            w2s_sb = wpool.tile([R*NG, 3*NG], f32, tag="w2s")
            nc.gpsimd.dma_start(w2s_sb[:], w2s[:])
            w2q_r = wpool.tile([R*NG, 3*NG], f32r, tag="w2qr")
            nc.vector.tensor_copy(w2q_r[:], w2q_sb[:])
            w2s_r = wpool.tile([R*NG, 3*NG], f32r, tag="w2sr")
            nc.vector.tensor_copy(w2s_r[:], w2s_sb[:])

            wm1t_sb = wpool.tile([3*TAIL_NG+1, R*TAIL_NG], f16, tag="wm1t")
            nc.gpsimd.dma_start(wm1t_sb[:], wm1t[:])
            w2qt_sb = wpool.tile([R*TAIL_NG, 3*TAIL_NG], f32, tag="w2qt")
            nc.gpsimd.dma_start(w2qt_sb[:], w2qt[:])
            w2st_sb = wpool.tile([R*TAIL_NG, 3*TAIL_NG], f32, tag="w2st")
            nc.gpsimd.dma_start(w2st_sb[:], w2st[:])
            w2qt_r = wpool.tile([R*TAIL_NG, 3*TAIL_NG], f32r, tag="w2qtr")
            nc.vector.tensor_copy(w2qt_r[:], w2qt_sb[:])
            w2st_r = wpool.tile([R*TAIL_NG, 3*TAIL_NG], f32r, tag="w2str")
            nc.vector.tensor_copy(w2st_r[:], w2st_sb[:])

            # pre-set the constant ones row in every X buffer slot (the
            # per-chunk DMA only writes rows 0..35, leaving row 36 intact)
            for _ in range(3):
                xt0 = xpool.tile([3*NG+1, N], f16, tag="X")
                nc.gpsimd.memset(xt0[0:1, :], 1.0)

            def chunk(src_slice, dst_slice, ngroups, m1w, qw, sw, p1p, p2p, tagsuf):
                KX = 3 * ngroups + 1
                RW = R * ngroups
                OW = 3 * ngroups
                gpb = ngroups // BPC
                xt = xpool.tile([KX, N], f16, tag="X" + tagsuf)
                if tagsuf:
                    nc.gpsimd.memset(xt[0:1, :], 1.0)
                for b in range(BPC):
                    for v in range(C):
                        nc.gpsimd.dma_start(
                            xt[1+b*3*gpb+v : 1+(b+1)*3*gpb : 3],
                            src_slice[b, v].rearrange("(g n) -> g n", n=N))

                p1 = p1p.tile([RW, N], f32, tag="P1" + tagsuf)
                nc.tensor.matmul(p1[:], m1w[:], xt[:], start=True, stop=True)

                s = spool.tile([RW, N], f32r, tag="S" + tagsuf)
                nc.scalar.square(s[:], p1[:])
                q = qpool.tile([RW, N], f32r, tag="Q" + tagsuf)
                nc.vector.tensor_mul(q[:], s[:], p1[:])

                p2 = p2p.tile([OW, N], f32, tag="P2" + tagsuf)
                nc.tensor.matmul(p2[:], qw[:], q[:], start=True, stop=False)
                nc.tensor.matmul(p2[:], sw[:], s[:], start=False, stop=True)

                o = opool.tile([OW, N], f32, tag="O" + tagsuf)
                nc.scalar.copy(o[:, 0:ACT_COLS], p2[:, 0:ACT_COLS])
                nc.vector.tensor_copy(o[:, ACT_COLS:N], p2[:, ACT_COLS:N])
                for b in range(BPC):
                    for c in range(C):
                        nc.gpsimd.dma_start(
                            dst_slice[b, c].rearrange("(g n) -> g n", n=N),
                            o[b*3*gpb+c : (b+1)*3*gpb : 3])

            def body():
                for k in range(FULL_CHUNKS):
                    lo, hi = k * GPB * N, (k + 1) * GPB * N
                    chunk(xs[:, :, lo:hi], y[:, :, lo:hi], NG,
                          wm1_sb, w2q_r, w2s_r, p1pool, p2pool, "")
                lo, hi = FULL_CHUNKS * GPB * N, HW
                chunk(xs[:, :, lo:hi], y[:, :, lo:hi], TAIL_NG,
                      wm1t_sb, w2qt_r, w2st_r, p1tpool, p2tpool, "T")

            if reps == 1:
                body()
            else:
                hint = (mybir.EngineType.PE, mybir.EngineType.Activation,
                        mybir.EngineType.DVE, mybir.EngineType.Pool)
                with tc.For_i(0, reps, 1, hint_engines=hint):
                    body()

    nc.compile()
    _NC_CACHE[reps] = nc
    return nc


def make_in_maps(x, weight, bias):
    cq, cs = _solve_coeffs(weight, bias)
    shared = {
        "wm1": _lhs1(NG),
        "w2q": _lhs2(cq, NG), "w2s": _lhs2(cs, NG),
        "wm1t": _lhs1(TAIL_NG),
        "w2qt": _lhs2(cq, TAIL_NG), "w2st": _lhs2(cs, TAIL_NG),
    }
    x = np.ascontiguousarray(np.asarray(x, np.float16)).reshape(B, C, HW)
    return [dict(shared, xs=x[i*BPC:(i+1)*BPC]) for i in range(NCORES)]


def kernel(x, weight, bias, degree=3, **_unused):
    assert int(degree) == 3, "kernel specialized for degree=3"
    nc = build_nc(reps=1)
    in_maps = make_in_maps(x, weight, bias)
    res = run_bass_kernel_spmd(nc, in_maps, core_ids=list(range(NCORES)))
    out = np.empty((B, C, HW), np.float32)
    for i in range(NCORES):
        out[i*BPC:(i+1)*BPC] = res.results[i]["y"]
    return out.reshape(B, C, H, W)


if __name__ == "__main__":
    rng = np.random.default_rng(0)
    x = rng.uniform(0, 1, size=(B, C, H, W)).astype(np.float32)
    weight = rng.normal(size=(19, 3)).astype(np.float32)
    bias = rng.normal(size=(3,)).astype(np.float32)
    got = kernel(x, weight, bias, 3)
    print("ran; out shape", got.shape)
